# revision 14
# baseline (speedup 1.0000x reference)
"""AttnBlock (channel attention over 64x64 maps) for Trainium2 — fp8 edition.

Data-parallel over batch: 16 batches, 2 per core on 8 NeuronCores.
Per batch [C=512, N=4096], with hnA = A*x (GroupNorm scale, folded), B the
GroupNorm shift:

  scores = q^T k with q = Wq(hnA+B)+bq factorizes through the Gram matrix
    G = hnA @ hnA^T  (C x C), so
    scores = (16Wq) G (16Wk)^T / 65536 + rank-1 corrections from channel
    rowsums (free from bn_stats means) — this kills the q/k projections
    entirely (two tiny C x C x C GEMMs replace two C x C x N passes).
  v / attn@v / out-proj run in fp8e4 DoubleRow (0.5 cyc/row) with
    residual-split operands: t ~ hi + lo, hi = fp8(t), lo = fp8(t - hi),
    giving ~bf16 accuracy at half (2/3 when both sides split) the PE cost.
  The x-residual is added on the PE via a 16*I identity matmul against the
    resident bf16 x, so no fp32 x reload and no extra DVE pass.

Weights are pre-split into fp8 hi/lo pairs (x16 scale) on the host.
All PSUM accumulation is fp32; softmax is max-subtracted (keeps e in fp8
range) with every power-of-2 scale folded into the softmax scale, rinv,
or the output scale.
"""

import sys

if "/opt/trn_rl_repo" not in sys.path:
    sys.path.insert(0, "/opt/trn_rl_repo")

import numpy as np

C = 512          # channels
N = 4096         # pixels (64*64)
BB = 2           # batches per core
P = 128          # partitions
CB = C // P      # 4 channel blocks
NT = N // P      # 32 pixel tiles of 128
NTH = 16         # pixel tiles per half (hnT buffer covers half the pixels)
NSL = 512        # pixel slice width (v / ef phases)
NS = N // NSL    # 8 pixel slices
GROUPS = 32
EPS = 1e-6
SCALE = float(C) ** -0.5
SC2 = SCALE / 65536.0
LN128 = float(np.log(128.0))

_NC_CACHE = {}
LAST_RESULT = None


def _build_nc():
    import concourse.bacc as bacc
    import concourse.tile as tile
    from concourse import mybir
    from concourse.bass import ts

    F32 = mybir.dt.float32
    BF16 = mybir.dt.bfloat16
    F8 = mybir.dt.float8e4
    AX = mybir.AxisListType
    AF = mybir.ActivationFunctionType
    OP = mybir.AluOpType
    DR = mybir.MatmulPerfMode.DoubleRow

    nc = bacc.Bacc(None, target_bir_lowering=False, num_swdge_queues=4)

    xsb_d = nc.dram_tensor("xsb", [BB, C, N], BF16, kind="ExternalInput")
    wqt16_d = nc.dram_tensor("wqt16", [C, C], BF16, kind="ExternalInput")
    wkt16_d = nc.dram_tensor("wkt16", [C, C], BF16, kind="ExternalInput")
    wvh_d = nc.dram_tensor("wvh", [C, C], F8, kind="ExternalInput")
    wvl_d = nc.dram_tensor("wvl", [C, C], F8, kind="ExternalInput")
    woh_d = nc.dram_tensor("woh", [C, C], F8, kind="ExternalInput")
    wol_d = nc.dram_tensor("wol", [C, C], F8, kind="ExternalInput")
    bq256_d = nc.dram_tensor("bq256", [C], F32, kind="ExternalInput")
    bk256_d = nc.dram_tensor("bk256", [C], F32, kind="ExternalInput")
    bv16_d = nc.dram_tensor("bv16", [C], F32, kind="ExternalInput")
    bo_d = nc.dram_tensor("bo", [C], F32, kind="ExternalInput")
    gamma_d = nc.dram_tensor("gamma", [C], F32, kind="ExternalInput")
    beta_d = nc.dram_tensor("beta", [C], F32, kind="ExternalInput")
    gfwd_d = nc.dram_tensor("gfwd", [P, CB, GROUPS], F32, kind="ExternalInput")
    gbwd_d = nc.dram_tensor("gbwd", [GROUPS, CB, P], F32, kind="ExternalInput")
    identbf_d = nc.dram_tensor("identbf", [P, P], BF16, kind="ExternalInput")
    ident16_d = nc.dram_tensor("ident16", [P, P], BF16, kind="ExternalInput")
    y_d = nc.dram_tensor("y", [BB, C, N], F32, kind="ExternalOutput")

    with tile.TileContext(nc) as tc:
        with (
            tc.tile_pool(name="singles", bufs=1) as sg,
            tc.tile_pool(name="sbp", bufs=1) as sbp,
            tc.tile_pool(name="psp", bufs=1, space="PSUM") as psp,
            tc.tile_pool(name="drp", bufs=1, space="DRAM") as drp,
        ):
            xbview = [xsb_d[b].rearrange("(cb p) n -> p cb n", p=P) for b in range(BB)]
            yview = [y_d[b].rearrange("(ob p) n -> p ob n", p=P) for b in range(BB)]
            st = [dict() for _ in range(BB)]  # per-batch tile state

            def emit_load(b):
                """bf16 x load. DMA only."""
                s = st[b]
                xbf = sbp.tile([P, CB, N], BF16, tag="xbf", bufs=1, name=f"xbf{b}")
                s["xbf"] = xbf
                for cb in range(CB):
                    nc.sync.dma_start(xbf[:, cb, :], xbview[b][:, cb, :])

            def emit_stats(b, split=False):
                """Per-channel [mean, E[x^2]] -> t."""
                s = st[b]
                xbf = s["xbf"]
                t = sbp.tile([P, CB, 2], F32, tag="t", bufs=2, name=f"t{b}")
                act_cbs = (0, 1) if split else ()
                bn_cbs = [cb for cb in range(CB) if cb not in act_cbs]
                stats = sbp.tile(
                    [P, CB, 8, 6], F32, tag="stats", bufs=2, name=f"st{b}"
                )
                mv = sbp.tile([P, CB, 2], F32, tag="mv", bufs=2, name=f"mv{b}")
                for cb in act_cbs:
                    # scratch shares the (not-yet-written) vh buffer
                    sq = sbp.tile([P, N], F32, tag="vh", bufs=1,
                                  name=f"sq{b}{cb}")
                    s1 = sbp.tile([P, 1], F32, tag="s1", bufs=2, name=f"s1{b}{cb}")
                    s2 = sbp.tile([P, 1], F32, tag="s2", bufs=2, name=f"s2{b}{cb}")
                    nc.scalar.activation(
                        sq, xbf[:, cb, :], AF.Copy, accum_out=s1
                    )
                    nc.scalar.activation(
                        sq, xbf[:, cb, :], AF.Square, accum_out=s2
                    )
                    nc.vector.tensor_scalar_mul(t[:, cb, 0:1], s1, 1.0 / N)
                    nc.vector.tensor_scalar_mul(t[:, cb, 1:2], s2, 1.0 / N)
                for cb in bn_cbs:
                    for j in range(8):
                        nc.vector.bn_stats(
                            stats[:, cb, j, :], xbf[:, cb, ts(j, 512)]
                        )
                    nc.vector.bn_aggr(mv[:, cb, :], stats[:, cb, :, :])
                for cb in bn_cbs:
                    nc.vector.tensor_mul(
                        t[:, cb, 1:2], mv[:, cb, 0:1], mv[:, cb, 0:1]
                    )
                    nc.vector.tensor_add(
                        t[:, cb, 1:2], t[:, cb, 1:2], mv[:, cb, 1:2]
                    )
                    nc.vector.tensor_copy(t[:, cb, 0:1], mv[:, cb, 0:1])
                s["t"] = t

            def emit_a2(b):
                """Group aggregation -> A (rstd*gamma), B (shift); derived
                per-batch constants: diag tiles, fp8/bf16 B copies, bias
                vectors for v and the scores rank-1 corrections."""
                s = st[b]
                t = s["t"]
                pg = psp.tile([GROUPS, 2], F32, tag="work", bufs=4, name=f"pg{b}")
                for cb in range(CB):
                    nc.tensor.matmul(
                        pg, gfwd[:, cb, :], t[:, cb, :],
                        start=(cb == 0), stop=(cb == CB - 1),
                    )
                gs = sbp.tile([GROUPS, 2], F32, tag="gs", bufs=2, name=f"gs{b}")
                pgs = sbp.tile([GROUPS, 2], F32, tag="pgs", bufs=2, name=f"pgs{b}")
                nc.vector.tensor_copy(pgs, pg)
                vtmp = sbp.tile([GROUPS, 1], F32, tag="vtmp", bufs=2, name=f"vt{b}")
                nc.vector.tensor_mul(vtmp, pgs[:, 0:1], pgs[:, 0:1])
                nc.vector.tensor_tensor(vtmp, pgs[:, 1:2], vtmp, op=OP.subtract)
                nc.vector.tensor_copy(gs[:, 0:1], pgs[:, 0:1])
                nc.scalar.activation(gs[:, 1:2], vtmp, AF.Sqrt, bias=eps_g)
                nc.vector.reciprocal(gs[:, 1:2], gs[:, 1:2])

                cst = sbp.tile([P, CB, 2], F32, tag="cst", bufs=2, name=f"cs{b}")
                for cb in range(CB):
                    pc = psp.tile([P, 2], F32, tag="work", bufs=4, name=f"pc{b}_{cb}")
                    nc.tensor.matmul(pc, gbwd[:, cb, :], gs, start=True, stop=True)
                    nc.vector.tensor_copy(cst[:, cb, :], pc)

                A_ = sbp.tile([P, CB], F32, tag="A_", bufs=2, name=f"A{b}")
                Bf = sbp.tile([P, CB], F32, tag="Bf", bufs=2, name=f"Bf{b}")
                Bb = sbp.tile([P, CB], BF16, tag="Bb", bufs=2, name=f"B{b}")
                B8 = sbp.tile([P, CB], F8, tag="B8", bufs=2, name=f"B8{b}")
                tmpB = sbp.tile([P, CB], F32, tag="tmpB", bufs=2, name=f"tB{b}")
                nc.vector.tensor_mul(A_, cst[:, :, 1], gam)
                nc.vector.tensor_mul(tmpB, cst[:, :, 0], A_)
                nc.vector.tensor_tensor(Bf, bet, tmpB, op=OP.subtract)
                nc.vector.tensor_copy(Bb, Bf)
                nc.vector.tensor_scalar_mul(B8, Bf, 64.0)
                s["A_"] = A_

                # diag tiles: D = diag(16*A) bf16, per cb
                Dt = sbp.tile([P, CB, P], BF16, tag="Dt", bufs=2, name=f"D{b}")
                s["Dt"] = Dt
                A16 = sbp.tile([P, CB], F32, tag="A16", bufs=2, name=f"A16{b}")
                nc.vector.tensor_scalar_mul(A16, A_, 16.0)
                for cb in range(CB):
                    nc.vector.tensor_scalar_mul(
                        Dt[:, cb, :], identbf, A16[:, cb : cb + 1]
                    )

                # v bias: bvb16 = 16*bv + 16*(Wv@B), via fp8 W pair and B8
                pb = psp.tile([1, C], F32, tag="work", bufs=4, name=f"pbv{b}")
                for wi, w in enumerate((wvh, wvl)):
                    for cb in range(CB):
                        nc.tensor.matmul(
                            pb, B8[:, cb : cb + 1], w[:, cb, :],
                            start=(wi == 0 and cb == 0),
                            stop=(wi == 1 and cb == CB - 1),
                        )
                bvrow = sbp.tile([1, C], F32, tag="bvrow", bufs=2, name=f"bvr{b}")
                nc.vector.tensor_scalar_mul(bvrow, pb, 1.0 / 64.0)
                nc.vector.tensor_add(bvrow, bvrow, bv16r)
                scr = drp.tile([C], F32, name=f"scrv{b}")
                nc.sync.dma_start(scr.rearrange("(a c) -> a c", a=1), bvrow)
                bvb16 = sbp.tile([P, CB], F32, tag="bvb16", bufs=2, name=f"bvb{b}")
                nc.sync.dma_start(bvb16, scr.rearrange("(cb p) -> p cb", p=P))
                s["bvb16"] = bvb16

                # scores rank-1 vectors (all at x256 scale):
                #   cq256 = 256*(Wq@B + bq), sq256 = 256*(Wq@rs), rs = A*N*mu
                rs16 = sbp.tile([P, CB], BF16, tag="rs16", bufs=2, name=f"rs{b}")
                rsf = sbp.tile([P, CB], F32, tag="rsf", bufs=2, name=f"rsf{b}")
                nc.vector.tensor_mul(rsf, A_, t[:, :, 0])
                nc.vector.tensor_scalar_mul(rs16, rsf, 16.0 * N)
                rows = {}
                for nm, wt, brow in (("q", wqt16, bq256r), ("k", wkt16, bk256r)):
                    pc1 = psp.tile([1, C], F32, tag="work", bufs=4,
                                   name=f"pc1{b}{nm}")
                    for cb in range(CB):
                        nc.tensor.matmul(
                            pc1, Bb[:, cb : cb + 1], wt[:, cb, :],
                            start=(cb == 0), stop=(cb == CB - 1),
                        )
                    crow = sbp.tile([1, C], BF16, tag=f"c{nm}row", bufs=2,
                                    name=f"c{nm}{b}")
                    tmpr = sbp.tile([1, C], F32, tag="tmpr", bufs=2,
                                    name=f"tr{b}{nm}")
                    nc.vector.tensor_scalar_mul(tmpr, pc1, 16.0)
                    nc.vector.tensor_add(crow, tmpr, brow)
                    rows[f"c{nm}"] = crow
                    ps1 = psp.tile([1, C], F32, tag="work", bufs=4,
                                   name=f"ps1{b}{nm}")
                    for cb in range(CB):
                        nc.tensor.matmul(
                            ps1, rs16[:, cb : cb + 1], wt[:, cb, :],
                            start=(cb == 0), stop=(cb == CB - 1),
                        )
                    srow = sbp.tile([1, C], BF16, tag=f"s{nm}row", bufs=2,
                                    name=f"s{nm}{b}")
                    nc.vector.tensor_copy(srow, ps1)
                    rows[f"s{nm}"] = srow
                # rhs1 = sk256 + N*ck256
                rhs1 = sbp.tile([1, C], BF16, tag="rhs1", bufs=2, name=f"rh{b}")
                nc.vector.tensor_scalar_mul(rhs1, rows["ck"], float(N))
                nc.vector.tensor_add(rhs1, rhs1, rows["sk"])
                s["cq"], s["sq"], s["ck"] = rows["cq"], rows["sq"], rows["ck"]
                s["rhs1"] = rhs1

            def emit_hnA(b):
                """hnA = A*x split to fp8 hi/lo (channel-major). DVE + GpSimd."""
                s = st[b]
                xbf, A_ = s["xbf"], s["A_"]
                hh = sbp.tile([P, CB, N], F8, tag="hh", bufs=1, name=f"hh{b}")
                hl = sbp.tile([P, CB, N], F8, tag="hl", bufs=1, name=f"hl{b}")
                s["hh"], s["hl"] = hh, hl
                for cb in range(CB):
                    nc.vector.tensor_scalar_mul(
                        hh[:, cb, :], xbf[:, cb, :], A_[:, cb : cb + 1]
                    )
                    nc.vector.scalar_tensor_tensor(
                        hl[:, cb, :], xbf[:, cb, :], A_[:, cb : cb + 1],
                        hh[:, cb, :], op0=OP.mult, op1=OP.subtract,
                    )

            def emit_gram(b):
                """hnT (pixel-major 16*hnA via PE diag matmul) -> Gram G
                -> T1 = wqt16^T G -> scores = T1^T wkt16 + rank-1 fixes."""
                s = st[b]
                xbf, Dt = s["xbf"], s["Dt"]
                hnT = sbp.tile([P, NTH, C], BF16, tag="hnT", bufs=1,
                               name=f"hnT{b}")
                pG = [
                    psp.tile([P, C], F32, tag="scores", bufs=4, name=f"pG{b}_{a}")
                    for a in range(CB)
                ]
                for half in range(2):
                    for ih in range(NTH):
                        i = half * NTH + ih
                        pT = psp.tile([P, C], F32, tag="work", bufs=4,
                                      name=f"pT{b}_{i}")
                        for cb in range(CB):
                            nc.tensor.matmul(
                                pT[:, ts(cb, P)], xbf[:, cb, ts(i, P)],
                                Dt[:, cb, :], start=True, stop=True,
                            )
                        nc.scalar.copy(hnT[:, ih, :], pT)
                    for ih in range(NTH):
                        i = half * NTH + ih
                        for a in range(CB):
                            nc.tensor.matmul(
                                pG[a], hnT[:, ih, ts(a, P)], hnT[:, ih, :],
                                start=(i == 0), stop=(i == NT - 1),
                            )
                Gb = sbp.tile([P, CB, C], BF16, tag="Gb", bufs=1, name=f"Gb{b}")
                for a in range(CB):
                    nc.scalar.copy(Gb[:, a, :], pG[a])
                # T1[o, d] = sum_c 16Wq[o,c] G[c,d]
                T1b = sbp.tile([P, CB, C], BF16, tag="T1b", bufs=1, name=f"T1{b}")
                for ocb in range(CB):
                    pT1 = psp.tile([P, C], F32, tag="work", bufs=4,
                                   name=f"pT1{b}_{ocb}")
                    for cb in range(CB):
                        nc.tensor.matmul(
                            pT1, wqt16[:, cb, ts(ocb, P)], Gb[:, cb, :],
                            start=(cb == 0), stop=(cb == CB - 1),
                        )
                    nc.scalar.copy(T1b[:, ocb, :], pT1)
                # transpose T1 -> T1T [d, o] (reuses Gb's buffer; Gb is dead)
                T1T = sbp.tile([P, CB, C], BF16, tag="Gb", bufs=1, name=f"TT{b}")
                for ocb in range(CB):
                    for db in range(CB):
                        nc.sync.dma_start(
                            T1T[:, db, ts(ocb, P)],
                            T1b[:, ocb, ts(db, P)],
                            transpose=True,
                        )
                # scores[o, e] = sum_d T1T[d, o] wkt16[d, e]  (+rank-1)
                scores = [
                    psp.tile([P, C], F32, tag="scores", bufs=4, name=f"sc{b}_{cb}")
                    for cb in range(CB)
                ]
                s["scores"] = scores
                cq, sq, ck, rhs1 = s["cq"], s["sq"], s["ck"], s["rhs1"]
                for ocb in range(CB):
                    for db in range(CB):
                        nc.tensor.matmul(
                            scores[ocb], T1T[:, db, ts(ocb, P)], wkt16[:, db, :],
                            start=(db == 0), stop=False,
                        )
                    nc.tensor.matmul(
                        scores[ocb], cq[:, ts(ocb, P)], rhs1,
                        start=False, stop=False,
                    )
                    nc.tensor.matmul(
                        scores[ocb], sq[:, ts(ocb, P)], ck,
                        start=False, stop=True,
                    )

            def emit_softmax(b):
                """Max-subtracted exp (x128), row sums -> rinv16."""
                s = st[b]
                scores = s["scores"]
                e_sb = sbp.tile([P, CB, C], BF16, tag="e", bufs=1, name=f"e{b}")
                rinv16 = sbp.tile([P, CB], F32, tag="rinv16", bufs=1,
                                  name=f"ri{b}")
                rmx = sbp.tile([P, CB], F32, tag="rmx", bufs=1, name=f"rm{b}")
                eb = sbp.tile([P, CB], F32, tag="eb", bufs=1, name=f"eb{b}")
                rsum = sbp.tile([P, CB], F32, tag="rsum", bufs=1, name=f"rs{b}")
                s["e"], s["rinv16"] = e_sb, rinv16
                for cb in range(CB):
                    nc.vector.reduce_max(
                        rmx[:, cb : cb + 1], scores[cb], axis=AX.X
                    )
                    nc.vector.tensor_scalar(
                        eb[:, cb : cb + 1], rmx[:, cb : cb + 1],
                        -SC2, LN128, op0=OP.mult, op1=OP.add,
                    )
                    nc.scalar.activation(
                        e_sb[:, cb, :], scores[cb], AF.Exp,
                        bias=eb[:, cb : cb + 1], scale=SC2,
                        accum_out=rsum[:, cb : cb + 1],
                    )
                    nc.vector.reciprocal(
                        rinv16[:, cb : cb + 1], rsum[:, cb : cb + 1]
                    )
                    nc.vector.tensor_scalar_mul(
                        rinv16[:, cb : cb + 1], rinv16[:, cb : cb + 1], 1.0 / 16.0
                    )

            def emit_t(b):
                """e -> eT (DMA transpose, bf16) -> eT8 (fp8)."""
                s = st[b]
                e_sb = s["e"]
                eT = sbp.tile([P, CB, C], BF16, tag="eT", bufs=1, name=f"eT{b}")
                eT8 = sbp.tile([P, CB, C], F8, tag="eT8", bufs=1, name=f"e8{b}")
                s["eT8"] = eT8
                for cb in range(CB):
                    for db in range(CB):
                        nc.sync.dma_start(
                            eT[:, db, ts(cb, P)],
                            e_sb[:, cb, ts(db, P)],
                            transpose=True,
                        )
                for cb in range(CB):
                    nc.gpsimd.tensor_copy(eT8[:, cb, :], eT[:, cb, :])

            def emit_v(b):
                """v' = 16v via fp8 DoubleRow (3-term split), hi/lo evict."""
                s = st[b]
                hh, hl, bvb16 = s["hh"], s["hl"], s["bvb16"]
                vh = sbp.tile([P, CB, N], F8, tag="vh", bufs=1, name=f"vh{b}")
                vl = sbp.tile([P, CB, N], F8, tag="vl", bufs=1, name=f"vl{b}")
                s["vh"], s["vl"] = vh, vl
                for nsl in range(NS):
                    for ob in range(CB):
                        pv = psp.tile([P, NSL], F32, tag="work", bufs=4,
                                      name=f"pv{b}{nsl}{ob}")
                        terms = ((wvh, hh), (wvh, hl), (wvl, hh))
                        nt = len(terms) * 2
                        k = 0
                        for w, h in terms:
                            for cbp in (0, 2):
                                nc.tensor.matmul(
                                    pv, w[:, cbp : cbp + 2, ts(ob, P)],
                                    h[:, cbp : cbp + 2, ts(nsl, NSL)],
                                    start=(k == 0), stop=(k == nt - 1),
                                    perf_mode=DR,
                                )
                                k += 1
                        nc.scalar.add(
                            vh[:, ob, ts(nsl, NSL)], pv, bvb16[:, ob : ob + 1]
                        )
                        nc.vector.scalar_tensor_tensor(
                            vl[:, ob, ts(nsl, NSL)], pv, bvb16[:, ob : ob + 1],
                            vh[:, ob, ts(nsl, NSL)],
                            op0=OP.add, op1=OP.subtract,
                        )

            def emit_ef(b):
                """attn@v (fp8 DR) -> ao hi/lo -> out-proj (fp8 DR)
                + 16*I x residual -> y."""
                s = st[b]
                eT8, vh, vl, rinv16 = s["eT8"], s["vh"], s["vl"], s["rinv16"]
                for nsl in range(NS):
                    xslb = sbp.tile([P, CB, NSL], BF16, tag="xslb", bufs=2,
                                    name=f"xsl{b}_{nsl}")
                    for cb in range(CB):
                        nc.gpsimd.dma_start(
                            xslb[:, cb, :], xbview[b][:, cb, ts(nsl, NSL)]
                        )
                    aoh = sbp.tile([P, CB, NSL], F8, tag="aoh", bufs=2,
                                   name=f"aoh{b}_{nsl}")
                    aol = sbp.tile([P, CB, NSL], F8, tag="aol", bufs=2,
                                   name=f"aol{b}_{nsl}")
                    for cb in range(CB):
                        pa = psp.tile([P, NSL], F32, tag="work", bufs=4,
                                      name=f"pa{b}{nsl}{cb}")
                        k = 0
                        for v8 in (vh, vl):
                            for dbp in (0, 2):
                                nc.tensor.matmul(
                                    pa, eT8[:, dbp : dbp + 2, ts(cb, P)],
                                    v8[:, dbp : dbp + 2, ts(nsl, NSL)],
                                    start=(k == 0), stop=(k == 3),
                                    perf_mode=DR,
                                )
                                k += 1
                        nc.scalar.mul(
                            aoh[:, cb, :], pa, rinv16[:, cb : cb + 1]
                        )
                        nc.vector.scalar_tensor_tensor(
                            aol[:, cb, :], pa, rinv16[:, cb : cb + 1],
                            aoh[:, cb, :], op0=OP.mult, op1=OP.subtract,
                        )
                    for ob in range(CB):
                        pf = psp.tile([P, NSL], F32, tag="work", bufs=4,
                                      name=f"pf{b}{nsl}{ob}")
                        k = 0
                        for w, a8 in ((woh, aoh), (wol, aoh), (woh, aol)):
                            for cbp in (0, 2):
                                nc.tensor.matmul(
                                    pf, w[:, cbp : cbp + 2, ts(ob, P)],
                                    a8[:, cbp : cbp + 2, :],
                                    start=(k == 0), stop=False,
                                    perf_mode=DR,
                                )
                                k += 1
                        nc.tensor.matmul(
                            pf, ident16, xslb[:, ob, :],
                            start=False, stop=True,
                        )
                        yt = sbp.tile([P, NSL], F32, tag="yt", bufs=2,
                                      name=f"yt{b}{nsl}{ob}")
                        nc.scalar.activation(
                            yt, pf, AF.Identity,
                            bias=bob[:, ob : ob + 1], scale=1.0 / 16.0,
                        )
                        nc.sync.dma_start(yview[b][:, ob, ts(nsl, NSL)], yt)

            # ---- prologue ----
            emit_load(0)
            # HAM warm-up: keep TensorE busy/clocked through the prologue.
            zsb = sg.tile([P, NSL], BF16, name="zsb")
            nc.gpsimd.memset(zsb, 0.0)
            pdum = psp.tile([P, NSL], F32, tag="work", bufs=4, name="pdum")
            for i in range(24):
                nc.tensor.matmul(
                    pdum, zsb[:, :P], zsb, start=(i == 0), stop=False
                )
            for cb in range(CB):
                nc.tensor.matmul(
                    pdum, st[0]["xbf"][:, cb, ts(0, P)], zsb,
                    start=False, stop=(cb == CB - 1),
                )
            dsb = sg.tile([1, 1], F32, name="dsb")
            nc.vector.tensor_copy(dsb, pdum[0:1, 0:1])
            dscr = drp.tile([1], F32, name="dscr")
            nc.sync.dma_start(dscr.rearrange("(a c) -> a c", a=1), dsb)
            # ---- constants, loaded once ----
            gfwd = sg.tile([P, CB, GROUPS], F32)
            nc.sync.dma_start(gfwd, gfwd_d[:])
            gbwd = sg.tile([GROUPS, CB, P], F32)
            nc.sync.dma_start(gbwd, gbwd_d[:])
            wqt16 = sg.tile([P, CB, C], BF16)
            nc.sync.dma_start(wqt16, wqt16_d[:].rearrange("(cb p) o -> p cb o", p=P))
            wkt16 = sg.tile([P, CB, C], BF16)
            nc.sync.dma_start(wkt16, wkt16_d[:].rearrange("(cb p) o -> p cb o", p=P))
            wvh = sg.tile([P, CB, C], F8)
            nc.sync.dma_start(wvh, wvh_d[:].rearrange("(cb p) o -> p cb o", p=P))
            wvl = sg.tile([P, CB, C], F8)
            nc.sync.dma_start(wvl, wvl_d[:].rearrange("(cb p) o -> p cb o", p=P))
            woh = sg.tile([P, CB, C], F8)
            nc.sync.dma_start(woh, woh_d[:].rearrange("(cb p) o -> p cb o", p=P))
            wol = sg.tile([P, CB, C], F8)
            nc.sync.dma_start(wol, wol_d[:].rearrange("(cb p) o -> p cb o", p=P))
            identbf = sg.tile([P, P], BF16)
            nc.sync.dma_start(identbf, identbf_d[:])
            ident16 = sg.tile([P, P], BF16)
            nc.sync.dma_start(ident16, ident16_d[:])
            gam = sg.tile([P, CB], F32)
            nc.sync.dma_start(gam, gamma_d[:].rearrange("(cb p) -> p cb", p=P))
            bet = sg.tile([P, CB], F32)
            nc.sync.dma_start(bet, beta_d[:].rearrange("(cb p) -> p cb", p=P))
            bob = sg.tile([P, CB], F32)
            nc.sync.dma_start(bob, bo_d[:].rearrange("(cb p) -> p cb", p=P))
            bq256r = sg.tile([1, C], F32)
            nc.sync.dma_start(bq256r, bq256_d[:].rearrange("(a c) -> a c", a=1))
            bk256r = sg.tile([1, C], F32)
            nc.sync.dma_start(bk256r, bk256_d[:].rearrange("(a c) -> a c", a=1))
            bv16r = sg.tile([1, C], F32)
            nc.sync.dma_start(bv16r, bv16_d[:].rearrange("(a c) -> a c", a=1))
            eps_g = sg.tile([GROUPS, 1], F32)
            nc.vector.memset(eps_g, EPS)

            emit_stats(0, split=True)
            emit_a2(0)
            for b in range(BB):
                emit_gram(b)
                emit_hnA(b)
                if b + 1 < BB:
                    emit_load(b + 1)
                emit_softmax(b)
                emit_t(b)
                if b + 1 < BB:
                    emit_stats(b + 1)
                emit_v(b)
                if b + 1 < BB:
                    emit_a2(b + 1)
                emit_ef(b)

    nc.finalize()
    return nc


def _get_nc():
    if "nc" not in _NC_CACHE:
        _NC_CACHE["nc"] = _build_nc()
    return _NC_CACHE["nc"]


def _make_consts():
    gfwd = np.zeros((P, CB, GROUPS), np.float32)
    gbwd = np.zeros((GROUPS, CB, P), np.float32)
    for cb in range(CB):
        for p in range(P):
            g = (cb * P + p) // 16
            gfwd[p, cb, g] = 1.0 / 16.0
            gbwd[g, cb, p] = 1.0
    return gfwd, gbwd


def kernel(x, gamma, beta, Wq, bq, Wk, bk, Wv, bv, Wo, bo):
    global LAST_RESULT
    from concourse.bass_utils import run_bass_kernel_spmd

    import ml_dtypes

    BF = ml_dtypes.bfloat16
    F8NP = ml_dtypes.float8_e4m3

    def wsplit(W):
        W16 = np.ascontiguousarray(np.asarray(W, np.float32).T * 16.0)
        Wh = W16.astype(F8NP)
        Wl = (W16 - Wh.astype(np.float32)).astype(F8NP)
        return np.ascontiguousarray(Wh), np.ascontiguousarray(Wl)

    x = np.ascontiguousarray(np.asarray(x, np.float32)).reshape(16, C, N)
    xb16 = np.ascontiguousarray(x.astype(BF))
    gfwd, gbwd = _make_consts()
    wvh, wvl = wsplit(Wv)
    woh, wol = wsplit(Wo)
    shared = {
        "wqt16": np.ascontiguousarray(
            (np.asarray(Wq, np.float32).T * 16.0).astype(BF)
        ),
        "wkt16": np.ascontiguousarray(
            (np.asarray(Wk, np.float32).T * 16.0).astype(BF)
        ),
        "wvh": wvh, "wvl": wvl, "woh": woh, "wol": wol,
        "bq256": np.ascontiguousarray(np.asarray(bq, np.float32) * 256.0),
        "bk256": np.ascontiguousarray(np.asarray(bk, np.float32) * 256.0),
        "bv16": np.ascontiguousarray(np.asarray(bv, np.float32) * 16.0),
        "bo": np.ascontiguousarray(np.asarray(bo, np.float32)),
        "gamma": np.ascontiguousarray(np.asarray(gamma, np.float32)),
        "beta": np.ascontiguousarray(np.asarray(beta, np.float32)),
        "gfwd": gfwd,
        "gbwd": gbwd,
        "identbf": np.ascontiguousarray(np.eye(P, dtype=np.float32).astype(BF)),
        "ident16": np.ascontiguousarray(
            (np.eye(P, dtype=np.float32) * 16.0).astype(BF)
        ),
    }
    in_maps = [
        dict(shared, xsb=np.ascontiguousarray(xb16[BB * i : BB * (i + 1)]))
        for i in range(8)
    ]
    nc = _get_nc()
    import os

    trace = os.environ.get("KERNEL_TRACE") == "1"
    res = run_bass_kernel_spmd(nc, in_maps, core_ids=list(range(8)), trace=trace)
    LAST_RESULT = res
    y = np.concatenate([r["y"] for r in res.results], axis=0)
    return y.reshape(16, C, 64, 64)


# revision 16
# speedup vs baseline: 1.3909x; 1.3909x over previous
"""AttnBlock (channel attention over 64x64 maps) for Trainium2 — Gram edition.

Data-parallel over batch: 16 batches, 2 per core on 8 NeuronCores.
Per batch [C=512, N=4096], hn = A*x + B (GroupNorm folded to per-channel
affine):

  scores = q^T k with q = Wq hn + bq factorizes through the Gram matrix
    G = (A*x) @ (A*x)^T  (C x C):
      scores = (16Wq) G (16Wk)^T / 65536  + rank-1 corrections
    where the corrections come from the per-channel rowsums (free from the
    bn_stats means) and the GroupNorm shift B. This replaces the q-proj,
    k-proj and scores passes (3 full C*C*N GEMMs + their PSUM evictions)
    with: a diagonal matmul building pixel-major hnT (16k cyc), the Gram
    GEMM (upper triangle + mirrored blocks, ~41k cyc), and two C*C*C GEMMs
    (8k cyc each). All bf16 with fp32 PSUM accumulation.
  v / attn@v / out-proj keep the proven bf16 layout: Wv's columns scaled
    by A so v projects straight from the resident bf16 x; softmax is
    max-subtracted; residual re-reads fp32 x slices.
Batches are software-pipelined; v-projection is split around the scores
GEMM to cover the T1-transpose DMA seam.
"""

import sys

if "/opt/trn_rl_repo" not in sys.path:
    sys.path.insert(0, "/opt/trn_rl_repo")

import numpy as np

C = 512          # channels
N = 4096         # pixels (64*64)
BB = 2           # batches per core
P = 128          # partitions
CB = C // P      # 4 channel blocks
NT = N // P      # 32 pixel tiles of 128
NTH = 8          # pixel tiles per hnT chunk
NSL = 512        # pixel slice width (v / ef phases)
NS = N // NSL    # 8 pixel slices
GROUPS = 32
EPS = 1e-6
SCALE = float(C) ** -0.5
SC2 = SCALE / 65536.0
LN128 = float(np.log(128.0))

_NC_CACHE = {}
LAST_RESULT = None


def _build_nc():
    import concourse.bacc as bacc
    import concourse.tile as tile
    from concourse import mybir
    from concourse.bass import ts

    F32 = mybir.dt.float32
    BF16 = mybir.dt.bfloat16
    AX = mybir.AxisListType
    AF = mybir.ActivationFunctionType
    OP = mybir.AluOpType

    nc = bacc.Bacc(None, target_bir_lowering=False, num_swdge_queues=4)

    xsb_d = nc.dram_tensor("xsb", [BB, C, N], BF16, kind="ExternalInput")
    wqt16_d = nc.dram_tensor("wqt16", [C, C], BF16, kind="ExternalInput")
    wkt16_d = nc.dram_tensor("wkt16", [C, C], BF16, kind="ExternalInput")
    wvt_d = nc.dram_tensor("wvtb", [C, C], BF16, kind="ExternalInput")
    wot_d = nc.dram_tensor("wotb", [C, C], BF16, kind="ExternalInput")
    bq256_d = nc.dram_tensor("bq256", [C], F32, kind="ExternalInput")
    bk256_d = nc.dram_tensor("bk256", [C], F32, kind="ExternalInput")
    bv_d = nc.dram_tensor("bv", [C], F32, kind="ExternalInput")
    bo_d = nc.dram_tensor("bo", [C], F32, kind="ExternalInput")
    gamma_d = nc.dram_tensor("gamma", [C], F32, kind="ExternalInput")
    beta_d = nc.dram_tensor("beta", [C], F32, kind="ExternalInput")
    gfwd_d = nc.dram_tensor("gfwd", [P, CB, GROUPS], F32, kind="ExternalInput")
    gbwd_d = nc.dram_tensor("gbwd", [GROUPS, CB, P], F32, kind="ExternalInput")
    identbf_d = nc.dram_tensor("identbf", [P, P], BF16, kind="ExternalInput")
    y_d = nc.dram_tensor("y", [BB, C, N], F32, kind="ExternalOutput")

    with tile.TileContext(nc) as tc:
        with (
            tc.tile_pool(name="singles", bufs=1) as sg,
            tc.tile_pool(name="sbp", bufs=1) as sbp,
            tc.tile_pool(name="psp", bufs=1, space="PSUM") as psp,
            tc.tile_pool(name="drp", bufs=1, space="DRAM") as drp,
        ):
            xbview = [xsb_d[b].rearrange("(cb p) n -> p cb n", p=P) for b in range(BB)]
            yview = [y_d[b].rearrange("(ob p) n -> p ob n", p=P) for b in range(BB)]
            st = [dict() for _ in range(BB)]  # per-batch tile state

            def emit_load(b):
                s = st[b]
                xbf = sbp.tile([P, CB, N], BF16, tag="xbf", bufs=2, name=f"xbf{b}")
                s["xbf"] = xbf
                for cb in range(CB):
                    nc.sync.dma_start(xbf[:, cb, :], xbview[b][:, cb, :])

            def emit_stats(b, split=False):
                """Per-channel [mean, E[x^2]] -> t."""
                s = st[b]
                xbf = s["xbf"]
                t = sbp.tile([P, CB, 2], F32, tag="t", bufs=2, name=f"t{b}")
                act_cbs = (0, 1) if split else ()
                bn_cbs = [cb for cb in range(CB) if cb not in act_cbs]
                stats = sbp.tile(
                    [P, CB, 8, 6], F32, tag="stats", bufs=2, name=f"st{b}"
                )
                mv = sbp.tile([P, CB, 2], F32, tag="mv", bufs=2, name=f"mv{b}")
                for cb in act_cbs:
                    # scratch shares the (not-yet-written) vfull buffer
                    sq = sbp.tile([P, N], F32, tag="vfull", bufs=1,
                                  name=f"sq{b}{cb}")
                    s1 = sbp.tile([P, 1], F32, tag="s1", bufs=2, name=f"s1{b}{cb}")
                    s2 = sbp.tile([P, 1], F32, tag="s2", bufs=2, name=f"s2{b}{cb}")
                    nc.scalar.activation(
                        sq, xbf[:, cb, :], AF.Copy, accum_out=s1
                    )
                    nc.scalar.activation(
                        sq, xbf[:, cb, :], AF.Square, accum_out=s2
                    )
                    nc.vector.tensor_scalar_mul(t[:, cb, 0:1], s1, 1.0 / N)
                    nc.vector.tensor_scalar_mul(t[:, cb, 1:2], s2, 1.0 / N)
                for cb in bn_cbs:
                    for j in range(8):
                        nc.vector.bn_stats(
                            stats[:, cb, j, :], xbf[:, cb, ts(j, 512)]
                        )
                    nc.vector.bn_aggr(mv[:, cb, :], stats[:, cb, :, :])
                for cb in bn_cbs:
                    nc.vector.tensor_mul(
                        t[:, cb, 1:2], mv[:, cb, 0:1], mv[:, cb, 0:1]
                    )
                    nc.vector.tensor_add(
                        t[:, cb, 1:2], t[:, cb, 1:2], mv[:, cb, 1:2]
                    )
                    nc.vector.tensor_copy(t[:, cb, 0:1], mv[:, cb, 0:1])
                s["t"] = t

            def emit_a2(b):
                """Group aggregation -> A, B; diag tiles; scaled Wv; biases;
                scores rank-1 correction vectors."""
                s = st[b]
                t = s["t"]
                pg = psp.tile([GROUPS, 2], F32, tag="work", bufs=4, name=f"pg{b}")
                for cb in range(CB):
                    nc.tensor.matmul(
                        pg, gfwd[:, cb, :], t[:, cb, :],
                        start=(cb == 0), stop=(cb == CB - 1),
                    )
                gs = sbp.tile([GROUPS, 2], F32, tag="gs", bufs=2, name=f"gs{b}")
                pgs = sbp.tile([GROUPS, 2], F32, tag="pgs", bufs=2, name=f"pgs{b}")
                nc.vector.tensor_copy(pgs, pg)
                vtmp = sbp.tile([GROUPS, 1], F32, tag="vtmp", bufs=2, name=f"vt{b}")
                nc.vector.tensor_mul(vtmp, pgs[:, 0:1], pgs[:, 0:1])
                nc.vector.tensor_tensor(vtmp, pgs[:, 1:2], vtmp, op=OP.subtract)
                nc.vector.tensor_copy(gs[:, 0:1], pgs[:, 0:1])
                nc.scalar.activation(gs[:, 1:2], vtmp, AF.Sqrt, bias=eps_g)
                nc.vector.reciprocal(gs[:, 1:2], gs[:, 1:2])

                cst = sbp.tile([P, CB, 2], F32, tag="cst", bufs=2, name=f"cs{b}")
                for cb in range(CB):
                    pc = psp.tile([P, 2], F32, tag="work", bufs=4, name=f"pc{b}_{cb}")
                    nc.tensor.matmul(pc, gbwd[:, cb, :], gs, start=True, stop=True)
                    nc.vector.tensor_copy(cst[:, cb, :], pc)

                A_ = sbp.tile([P, CB], F32, tag="A_", bufs=2, name=f"A{b}")
                Bb = sbp.tile([P, CB], BF16, tag="Bb", bufs=2, name=f"B{b}")
                tmpB = sbp.tile([P, CB], F32, tag="tmpB", bufs=2, name=f"tB{b}")
                nc.vector.tensor_mul(A_, cst[:, :, 1], gam)
                nc.vector.tensor_mul(tmpB, cst[:, :, 0], A_)
                nc.vector.tensor_tensor(Bb, bet, tmpB, op=OP.subtract)

                # diag tiles D = diag(16*A) for the pixel-major hnT build
                Dt = sbp.tile([P, CB, P], BF16, tag="Dt", bufs=2, name=f"D{b}")
                s["Dt"] = Dt
                A16 = sbp.tile([P, CB], F32, tag="A16", bufs=2, name=f"A16{b}")
                nc.vector.tensor_scalar_mul(A16, A_, 16.0)
                for cb in range(CB):
                    nc.vector.tensor_scalar_mul(
                        Dt[:, cb, :], identbf, A16[:, cb : cb + 1]
                    )

                # Wv columns scaled by A
                wv_p = sbp.tile([P, CB, C], BF16, tag="wv_p", bufs=2, name=f"wv{b}")
                s["wv_p"] = wv_p
                for cb in range(CB):
                    nc.vector.tensor_scalar_mul(
                        wv_p[:, cb, :], wvt[:, cb, :], A_[:, cb : cb + 1]
                    )

                # v bias: bvb = bv + Wv@B, via DRAM round-trip to [P, CB]
                pb = psp.tile([1, C], F32, tag="work", bufs=4, name=f"pbv{b}")
                for cb in range(CB):
                    nc.tensor.matmul(
                        pb, Bb[:, cb : cb + 1], wvt[:, cb, :],
                        start=(cb == 0), stop=(cb == CB - 1),
                    )
                bvrow = sbp.tile([1, C], F32, tag="bvrow", bufs=2, name=f"bvr{b}")
                nc.vector.tensor_add(bvrow, pb, bvv)
                scr = drp.tile([C], F32, name=f"scrv{b}")
                nc.sync.dma_start(scr.rearrange("(a c) -> a c", a=1), bvrow)
                bvb = sbp.tile([P, CB], F32, tag="bvb", bufs=2, name=f"bvb{b}")
                nc.sync.dma_start(bvb, scr.rearrange("(cb p) -> p cb", p=P))
                s["bvb"] = bvb

                # scores rank-1 vectors (x256 scale):
                #   cq256 = 256*(Wq@B + bq), sq256 = 256*(Wq@rs), rs = A*N*mu
                rs16 = sbp.tile([P, CB], BF16, tag="rs16", bufs=2, name=f"rs{b}")
                rsf = sbp.tile([P, CB], F32, tag="rsf", bufs=2, name=f"rsf{b}")
                nc.vector.tensor_mul(rsf, A_, t[:, :, 0])
                nc.vector.tensor_scalar_mul(rs16, rsf, 16.0 * N)
                rows = {}
                for nm, wt, brow in (("q", wqt16, bq256r), ("k", wkt16, bk256r)):
                    pc1 = psp.tile([1, C], F32, tag="work", bufs=4,
                                   name=f"pc1{b}{nm}")
                    for cb in range(CB):
                        nc.tensor.matmul(
                            pc1, Bb[:, cb : cb + 1], wt[:, cb, :],
                            start=(cb == 0), stop=(cb == CB - 1),
                        )
                    crow = sbp.tile([1, C], BF16, tag=f"c{nm}row", bufs=2,
                                    name=f"c{nm}{b}")
                    tmpr = sbp.tile([1, C], F32, tag="tmpr", bufs=2,
                                    name=f"tr{b}{nm}")
                    nc.vector.tensor_scalar_mul(tmpr, pc1, 16.0)
                    nc.vector.tensor_add(crow, tmpr, brow)
                    rows[f"c{nm}"] = crow
                    ps1 = psp.tile([1, C], F32, tag="work", bufs=4,
                                   name=f"ps1{b}{nm}")
                    for cb in range(CB):
                        nc.tensor.matmul(
                            ps1, rs16[:, cb : cb + 1], wt[:, cb, :],
                            start=(cb == 0), stop=(cb == CB - 1),
                        )
                    srow = sbp.tile([1, C], BF16, tag=f"s{nm}row", bufs=2,
                                    name=f"s{nm}{b}")
                    nc.vector.tensor_copy(srow, ps1)
                    rows[f"s{nm}"] = srow
                rhs1 = sbp.tile([1, C], BF16, tag="rhs1", bufs=2, name=f"rh{b}")
                nc.vector.tensor_scalar_mul(rhs1, rows["ck"], float(N))
                nc.vector.tensor_add(rhs1, rhs1, rows["sk"])
                s["cq"], s["sq"], s["ck"] = rows["cq"], rows["sq"], rows["ck"]
                s["rhs1"] = rhs1

            def emit_gram(b):
                """hnT (pixel-major 16*A*x via PE diag matmul) -> Gram
                (upper triangle + mirrored blocks) -> T1 = (16Wq)^T G ->
                T1T (DMA transpose)."""
                s = st[b]
                xbf, Dt = s["xbf"], s["Dt"]
                hnT = sbp.tile([P, NTH, C], BF16, tag="hnT", bufs=1,
                               name=f"hnT{b}")
                pG = [
                    psp.tile([P, C - a * P], F32, tag="scores", bufs=4,
                             name=f"pG{b}_{a}")
                    for a in range(CB)
                ]
                for half in range(NT // NTH):
                    for ih in range(NTH):
                        i = half * NTH + ih
                        pT = psp.tile([P, C], F32, tag="work", bufs=4,
                                      name=f"pT{b}_{i}")
                        for cb in range(CB):
                            nc.tensor.matmul(
                                pT[:, ts(cb, P)], xbf[:, cb, ts(i, P)],
                                Dt[:, cb, :], start=True, stop=True,
                            )
                        nc.scalar.copy(hnT[:, ih, :], pT)
                    for ih in range(NTH):
                        i = half * NTH + ih
                        for a in range(CB):
                            nc.tensor.matmul(
                                pG[a], hnT[:, ih, ts(a, P)],
                                hnT[:, ih, a * P :],
                                start=(i == 0), stop=(i == NT - 1),
                            )
                Gb = sbp.tile([P, CB, C], BF16, tag="Gb", bufs=1, name=f"Gb{b}")
                for a in range(CB):
                    nc.scalar.copy(Gb[:, a, a * P :], pG[a])
                # mirror the 6 sub-diagonal blocks: G[b,a] = G[a,b]^T
                for a in range(CB):
                    for bb2 in range(a + 1, CB):
                        nc.sync.dma_start(
                            Gb[:, bb2, ts(a, P)],
                            Gb[:, a, ts(bb2, P)],
                            transpose=True,
                        )
                # T1[o, d] = sum_c 16Wq[o,c] G[c,d]
                T1b = sbp.tile([P, CB, C], BF16, tag="T1b", bufs=1, name=f"T1{b}")
                for ocb in range(CB):
                    pT1 = psp.tile([P, C], F32, tag="work", bufs=4,
                                   name=f"pT1{b}_{ocb}")
                    for cb in range(CB):
                        nc.tensor.matmul(
                            pT1, wqt16[:, cb, ts(ocb, P)], Gb[:, cb, :],
                            start=(cb == 0), stop=(cb == CB - 1),
                        )
                    nc.scalar.copy(T1b[:, ocb, :], pT1)
                # transpose T1 -> T1T [d, o] (reuses Gb's buffer next batch)
                T1T = sbp.tile([P, CB, C], BF16, tag="Gb", bufs=1, name=f"TT{b}")
                s["T1T"] = T1T
                for ocb in range(CB):
                    for db in range(CB):
                        nc.sync.dma_start(
                            T1T[:, db, ts(ocb, P)],
                            T1b[:, ocb, ts(db, P)],
                            transpose=True,
                        )

            def emit_scores(b):
                """scores[o, e] = sum_d T1T[d, o] wkt16[d, e] + rank-1."""
                s = st[b]
                T1T = s["T1T"]
                cq, sq, ck, rhs1 = s["cq"], s["sq"], s["ck"], s["rhs1"]
                scores = [
                    psp.tile([P, C], F32, tag="scores", bufs=4, name=f"sc{b}_{cb}")
                    for cb in range(CB)
                ]
                s["scores"] = scores
                for ocb in range(CB):
                    for db in range(CB):
                        nc.tensor.matmul(
                            scores[ocb], T1T[:, db, ts(ocb, P)], wkt16[:, db, :],
                            start=(db == 0), stop=False,
                        )
                    nc.tensor.matmul(
                        scores[ocb], cq[:, ts(ocb, P)], rhs1,
                        start=False, stop=False,
                    )
                    nc.tensor.matmul(
                        scores[ocb], sq[:, ts(ocb, P)], ck,
                        start=False, stop=True,
                    )

            def emit_softmax(b):
                """Max-subtracted exp (x128), row sums -> rinv."""
                s = st[b]
                scores = s["scores"]
                e_sb = sbp.tile([P, CB, C], BF16, tag="e", bufs=1, name=f"e{b}")
                rinv = sbp.tile([P, CB], F32, tag="rinv", bufs=1, name=f"ri{b}")
                rmx = sbp.tile([P, CB], F32, tag="rmx", bufs=1, name=f"rm{b}")
                eb = sbp.tile([P, CB], F32, tag="eb", bufs=1, name=f"eb{b}")
                rsum = sbp.tile([P, CB], F32, tag="rsum", bufs=1, name=f"rs{b}")
                s["e"], s["rinv"] = e_sb, rinv
                for cb in range(CB):
                    nc.vector.reduce_max(
                        rmx[:, cb : cb + 1], scores[cb], axis=AX.X
                    )
                    nc.vector.tensor_scalar(
                        eb[:, cb : cb + 1], rmx[:, cb : cb + 1],
                        -SC2, LN128, op0=OP.mult, op1=OP.add,
                    )
                    nc.scalar.activation(
                        e_sb[:, cb, :], scores[cb], AF.Exp,
                        bias=eb[:, cb : cb + 1], scale=SC2,
                        accum_out=rsum[:, cb : cb + 1],
                    )
                    nc.vector.reciprocal(
                        rinv[:, cb : cb + 1], rsum[:, cb : cb + 1]
                    )

            def emit_t(b):
                """Transpose e -> eT via DMA transpose (bf16)."""
                s = st[b]
                e_sb = s["e"]
                eT = sbp.tile([P, CB, C], BF16, tag="eT", bufs=1, name=f"eT{b}")
                s["eT"] = eT
                for cb in range(CB):
                    for db in range(CB):
                        nc.sync.dma_start(
                            eT[:, db, ts(cb, P)],
                            e_sb[:, cb, ts(db, P)],
                            transpose=True,
                        )

            def emit_v(b, nsls):
                """v projection for the given pixel slices."""
                s = st[b]
                xbf, wv_p, bvb = s["xbf"], s["wv_p"], s["bvb"]
                if "vfull" not in s:
                    s["vfull"] = sbp.tile([P, CB, N], BF16, tag="vfull", bufs=1,
                                          name=f"v{b}")
                vfull = s["vfull"]
                for nsl in nsls:
                    for ob in range(CB):
                        pv = psp.tile([P, NSL], F32, tag="work", bufs=4,
                                      name=f"pv{b}{nsl}{ob}")
                        for cb in range(CB):
                            nc.tensor.matmul(
                                pv, wv_p[:, cb, ts(ob, P)],
                                xbf[:, cb, ts(nsl, NSL)],
                                start=(cb == 0), stop=(cb == CB - 1),
                            )
                        if (nsl * CB + ob) % 4 == 3:
                            nc.vector.tensor_scalar_add(
                                vfull[:, ob, ts(nsl, NSL)], pv,
                                bvb[:, ob : ob + 1],
                            )
                        else:
                            nc.scalar.add(
                                vfull[:, ob, ts(nsl, NSL)], pv,
                                bvb[:, ob : ob + 1],
                            )

            def emit_ef(b, early_free=False):
                s = st[b]
                eT, vfull, rinv = s["eT"], s["vfull"], s["rinv"]
                for nsl in range(NS):
                    pf_tag = "work" if (early_free and nsl >= NS - 1) else "scores"
                    xsl = sbp.tile([P, CB, NSL], BF16, tag="xsl", bufs=2,
                                   name=f"xs{b}_{nsl}")
                    for cb in range(CB):
                        nc.gpsimd.dma_start(
                            xsl[:, cb, :], xbview[b][:, cb, ts(nsl, NSL)]
                        )
                    ao = sbp.tile([P, CB, NSL], BF16, tag="ao", bufs=2,
                                  name=f"ao{b}_{nsl}")
                    for cb in range(CB):
                        pa = psp.tile([P, NSL], F32, tag="work", bufs=4,
                                      name=f"pa{b}{nsl}{cb}")
                        for db in range(CB):
                            nc.tensor.matmul(
                                pa, eT[:, db, ts(cb, P)],
                                vfull[:, db, ts(nsl, NSL)],
                                start=(db == 0), stop=(db == CB - 1),
                            )
                        nc.scalar.mul(ao[:, cb, :], pa, rinv[:, cb : cb + 1])

                    for ob in range(CB):
                        pf = psp.tile([P, NSL], F32, tag=pf_tag, bufs=4,
                                      name=f"pf{b}{nsl}{ob}")
                        for cb in range(CB):
                            nc.tensor.matmul(
                                pf, wot[:, cb, ts(ob, P)], ao[:, cb, :],
                                start=(cb == 0), stop=(cb == CB - 1),
                            )
                        yt = sbp.tile([P, NSL], F32, tag="yt", bufs=3,
                                      name=f"yt{b}{nsl}{ob}")
                        nc.vector.scalar_tensor_tensor(
                            yt, pf, bob[:, ob : ob + 1], xsl[:, ob, :],
                            op0=OP.add, op1=OP.add,
                        )
                        nc.sync.dma_start(yview[b][:, ob, ts(nsl, NSL)], yt)

            # ---- prologue ----
            emit_load(0)
            # HAM warm-up: keep TensorE busy/clocked through the prologue.
            zsb = sg.tile([P, NSL], BF16, name="zsb")
            nc.gpsimd.memset(zsb, 0.0)
            pdum = psp.tile([P, NSL], F32, tag="work", bufs=4, name="pdum")
            for i in range(24):
                nc.tensor.matmul(
                    pdum, zsb[:, :P], zsb, start=(i == 0), stop=False
                )
            for cb in range(CB):
                nc.tensor.matmul(
                    pdum, st[0]["xbf"][:, cb, ts(0, P)], zsb,
                    start=False, stop=(cb == CB - 1),
                )
            dsb = sg.tile([1, 1], F32, name="dsb")
            nc.vector.tensor_copy(dsb, pdum[0:1, 0:1])
            dscr = drp.tile([1], F32, name="dscr")
            nc.sync.dma_start(dscr.rearrange("(a c) -> a c", a=1), dsb)
            # ---- constants, loaded once ----
            gfwd = sg.tile([P, CB, GROUPS], F32)
            nc.sync.dma_start(gfwd, gfwd_d[:])
            gbwd = sg.tile([GROUPS, CB, P], F32)
            nc.sync.dma_start(gbwd, gbwd_d[:])
            wqt16 = sg.tile([P, CB, C], BF16)
            nc.sync.dma_start(wqt16, wqt16_d[:].rearrange("(cb p) o -> p cb o", p=P))
            wkt16 = sg.tile([P, CB, C], BF16)
            nc.sync.dma_start(wkt16, wkt16_d[:].rearrange("(cb p) o -> p cb o", p=P))
            wvt = sg.tile([P, CB, C], BF16)
            nc.sync.dma_start(wvt, wvt_d[:].rearrange("(cb p) o -> p cb o", p=P))
            wot = sg.tile([P, CB, C], BF16)
            nc.sync.dma_start(wot, wot_d[:].rearrange("(cb p) o -> p cb o", p=P))
            identbf = sg.tile([P, P], BF16)
            nc.sync.dma_start(identbf, identbf_d[:])
            gam = sg.tile([P, CB], F32)
            nc.sync.dma_start(gam, gamma_d[:].rearrange("(cb p) -> p cb", p=P))
            bet = sg.tile([P, CB], F32)
            nc.sync.dma_start(bet, beta_d[:].rearrange("(cb p) -> p cb", p=P))
            bob = sg.tile([P, CB], F32)
            nc.sync.dma_start(bob, bo_d[:].rearrange("(cb p) -> p cb", p=P))
            bq256r = sg.tile([1, C], F32)
            nc.sync.dma_start(bq256r, bq256_d[:].rearrange("(a c) -> a c", a=1))
            bk256r = sg.tile([1, C], F32)
            nc.sync.dma_start(bk256r, bk256_d[:].rearrange("(a c) -> a c", a=1))
            bvv = sg.tile([1, C], F32)
            nc.sync.dma_start(bvv, bv_d[:].rearrange("(a c) -> a c", a=1))
            eps_g = sg.tile([GROUPS, 1], F32)
            nc.vector.memset(eps_g, EPS)

            emit_stats(0, split=True)
            emit_a2(0)
            for b in range(BB):
                emit_gram(b)
                if b + 1 < BB:
                    emit_load(b + 1)
                emit_v(b, range(0, 6))
                emit_scores(b)
                emit_softmax(b)
                emit_t(b)
                if b + 1 < BB:
                    emit_stats(b + 1)
                emit_v(b, range(6, NS))
                if b + 1 < BB:
                    emit_a2(b + 1)
                emit_ef(b, early_free=(b + 1 < BB))

    nc.finalize()
    return nc


def _get_nc():
    if "nc" not in _NC_CACHE:
        _NC_CACHE["nc"] = _build_nc()
    return _NC_CACHE["nc"]


def _make_consts():
    gfwd = np.zeros((P, CB, GROUPS), np.float32)
    gbwd = np.zeros((GROUPS, CB, P), np.float32)
    for cb in range(CB):
        for p in range(P):
            g = (cb * P + p) // 16
            gfwd[p, cb, g] = 1.0 / 16.0
            gbwd[g, cb, p] = 1.0
    return gfwd, gbwd


def kernel(x, gamma, beta, Wq, bq, Wk, bk, Wv, bv, Wo, bo):
    global LAST_RESULT
    from concourse.bass_utils import run_bass_kernel_spmd

    import ml_dtypes

    BF = ml_dtypes.bfloat16
    x = np.ascontiguousarray(np.asarray(x, np.float32)).reshape(16, C, N)
    xb16 = np.ascontiguousarray(x.astype(BF))
    gfwd, gbwd = _make_consts()
    shared = {
        "wqt16": np.ascontiguousarray(
            (np.asarray(Wq, np.float32).T * 16.0).astype(BF)
        ),
        "wkt16": np.ascontiguousarray(
            (np.asarray(Wk, np.float32).T * 16.0).astype(BF)
        ),
        "wvtb": np.ascontiguousarray(np.asarray(Wv, np.float32).T.astype(BF)),
        "wotb": np.ascontiguousarray(np.asarray(Wo, np.float32).T.astype(BF)),
        "bq256": np.ascontiguousarray(np.asarray(bq, np.float32) * 256.0),
        "bk256": np.ascontiguousarray(np.asarray(bk, np.float32) * 256.0),
        "bv": np.ascontiguousarray(np.asarray(bv, np.float32)),
        "bo": np.ascontiguousarray(np.asarray(bo, np.float32)),
        "gamma": np.ascontiguousarray(np.asarray(gamma, np.float32)),
        "beta": np.ascontiguousarray(np.asarray(beta, np.float32)),
        "gfwd": gfwd,
        "gbwd": gbwd,
        "identbf": np.ascontiguousarray(np.eye(P, dtype=np.float32).astype(BF)),
    }
    in_maps = [
        dict(shared, xsb=np.ascontiguousarray(xb16[BB * i : BB * (i + 1)]))
        for i in range(8)
    ]
    nc = _get_nc()
    import os

    trace = os.environ.get("KERNEL_TRACE") == "1"
    res = run_bass_kernel_spmd(nc, in_maps, core_ids=list(range(8)), trace=trace)
    LAST_RESULT = res
    y = np.concatenate([r["y"] for r in res.results], axis=0)
    return y.reshape(16, C, 64, 64)
